# revision 39
# baseline (speedup 1.0000x reference)
"""AConnect noisy-weight layer on 8 TRN2 NeuronCores.

Z[b] = X[b] @ (W * Werr[idx[b]]) + bias * Berr[idx[b]]

Strategy: data-parallel over batch (32 batch elements per core), W/bias/
Werr/Berr replicated. Per batch element the core fetches the 1MB noise
slice Werr[idx[b]] with a hardware-DGE DMA whose DRAM offset comes from a
register (idx loaded from SBUF, scaled on the sync engine) — this keeps
descriptor generation in RTL and off the gpsimd Q7. The slice is
multiplied elementwise with W on the vector engine (cast to bf16 on
write), then contracted with X[b] via 4 accumulating PE matmuls into a
[1, 512] PSUM tile that the scalar engine pre-loaded with the noisy bias
row. The scalar engine also evacuates finished rows; four batched DMAs
write the output. No collectives needed; host concatenates the 8 shards.
"""

import numpy as np

import concourse.bass as bass
import concourse.mybir as mybir
from concourse.tile import TileContext
from concourse.masks import make_identity
from concourse.bass_utils import run_bass_kernel_spmd

P = 128
N_CORES = 8
B = 256
IN = 512
OUT = 512
POOL = 1000

F32 = mybir.dt.float32
BF16 = mybir.dt.bfloat16
I32 = mybir.dt.int32


def _legalize_waits(nc):
    """Move surplus sync waits onto InstEventSemaphore carriers.

    TRN2 engine instructions encode a single sync wait (walrus codegen
    fails with "Too many sync wait commands" otherwise), but the Tile
    scheduler sometimes attaches several. Hoist all but one onto fresh
    event-semaphore instructions inserted immediately before the target
    in its block — same engine, so they execute in order and the target
    only issues once every hoisted wait has been satisfied.
    """
    n = 0
    for f in nc.m.functions:
        for blk in f.blocks:
            new_insts = []
            for inst in blk.instructions:
                si = inst.sync_info
                waits = list(si.on_wait) if si is not None else []
                if len(waits) > 1:
                    for i, w in enumerate(waits[:-1]):
                        n += 1
                        new_insts.append(mybir.InstEventSemaphore(
                            name=f"{inst.name}-esw{i}",
                            engine=inst.engine,
                            ins=[], outs=[],
                            sync_info=mybir.SyncInfo(on_wait=[w], on_update=[]),
                        ))
                    inst.sync_info = mybir.SyncInfo(
                        on_wait=[waits[-1]], on_update=list(si.on_update))
                new_insts.append(inst)
            blk.instructions = new_insts
    return n


def build_nc(b_shard=B // N_CORES, in_=IN, out=OUT, pool=POOL,
             werr_bufs=6, tmp_bufs=4, zpsum_bufs=5, gp_cols=0,
             swdge_every=3, legalize=True):
    """Build the per-core Bass graph (SPMD; same graph on all 8 cores)."""
    nj = in_ // P            # 4 sub-rows of the contraction dim per partition
    chunk = nj * out         # 2048 free elements per partition of a Werr slice

    nc = bass.Bass()
    x_d = nc.declare_dram_parameter("X", [b_shard, in_], F32, isOutput=False)
    w_d = nc.declare_dram_parameter("W", [P, chunk], F32, isOutput=False)
    bias_d = nc.declare_dram_parameter("bias", [1, out], F32, isOutput=False)
    werr_d = nc.declare_dram_parameter("Werr", [pool * P, chunk], F32, isOutput=False)
    berr_d = nc.declare_dram_parameter("Berr", [pool, out], F32, isOutput=False)
    idx_d = nc.declare_dram_parameter("idx", [b_shard, 1], I32, isOutput=False)
    idxt_d = nc.declare_dram_parameter("idxT", [1, b_shard], I32, isOutput=False)
    out_d = nc.declare_dram_parameter("out", [b_shard, out], F32, isOutput=True)
    # DRAM staging buffer used to re-layout the noisy bias from [b, out]
    # (one row per partition) to [1, b*out] (all rows on partition 0).
    nbstage_d = nc.dram_tensor("nbstage", [b_shard, out], BF16)

    with TileContext(nc) as tc:
        with (
            tc.tile_pool(name="const", bufs=1) as cpool,
            tc.tile_pool(name="werr", bufs=werr_bufs) as wpool,
            tc.tile_pool(name="tmp", bufs=tmp_bufs) as tpool,
            tc.tile_pool(name="spsum", bufs=1, space="PSUM") as spp,
            tc.tile_pool(name="zpsum", bufs=zpsum_bufs, space="PSUM") as zpp,
        ):
            # ---------------- constants / setup ----------------
            # idxt first — the gather offsets depend on it and nothing else,
            # so the first gathers can issue while the rest of setup loads.
            idxt_sb = cpool.tile([1, b_shard], I32)
            nc.gpsimd.dma_start(out=idxt_sb[:], in_=idxt_d[:])
            idx_sb = cpool.tile([b_shard, 1], I32)
            nc.gpsimd.dma_start(out=idx_sb[:], in_=idx_d[:])
            # Setup loads ride SWDGE so the HWDGE queues stay clear for the
            # gathers (the 1MB W load would otherwise delay the pipeline).
            w_sb = cpool.tile([P, chunk], F32)
            nc.gpsimd.dma_start(out=w_sb[:], in_=w_d[:])
            x_pl = cpool.tile([b_shard, in_], F32)
            nc.gpsimd.dma_start(out=x_pl[:], in_=x_d[:])
            bias_sb = cpool.tile([1, out], F32)
            nc.gpsimd.dma_start(out=bias_sb[:], in_=bias_d[:])
            # bf16 W for the SWDGE-gathered (bf16) iterations.
            w_bf = cpool.tile([P, chunk], BF16)
            nc.vector.tensor_copy(out=w_bf[:], in_=w_sb[:])

            # Identity for the PE transposes, re-materialized by DVE.
            ident_g = cpool.tile([P, P], F32)
            make_identity(nc, ident_g[:])
            ident = cpool.tile([P, P], F32)
            nc.vector.tensor_copy(out=ident[:], in_=ident_g[:])

            ones_row = cpool.tile([1, P], F32)
            nc.vector.memset(ones_row[:], 1.0)

            # DVE-owned copies of DMA-loaded tiles consumed by PE.
            x_v = cpool.tile([b_shard, in_], F32)
            nc.vector.tensor_copy(out=x_v[:], in_=x_pl[:])
            bias_v = cpool.tile([1, out], F32)
            nc.vector.tensor_copy(out=bias_v[:], in_=bias_sb[:])

            # Row indices for the SWDGE (bf16-casting) gathers:
            # rows[p, b] = idx[b]*128 + p into the [pool*128, chunk] view.
            idxt_f = cpool.tile([1, b_shard], F32)
            nc.vector.tensor_copy(out=idxt_f[:], in_=idxt_sb[:])
            ps_bc = spp.tile([P, b_shard], F32, space="PSUM")
            nc.tensor.matmul(ps_bc[:], lhsT=ones_row[:], rhs=idxt_f[:],
                             start=True, stop=True)
            rows = cpool.tile([P, b_shard], I32)
            nc.vector.tensor_copy(out=rows[:], in_=ps_bc[:])
            nc.vector.tensor_scalar_mul(rows[:], rows[:], P)
            iota_p = cpool.tile([P, 1], I32)
            nc.gpsimd.iota(iota_p[:], pattern=[[0, 1]], base=0,
                           channel_multiplier=1)
            nc.vector.tensor_tensor(out=rows[:], in0=rows[:],
                                    in1=iota_p[:].to_broadcast([P, b_shard]),
                                    op=mybir.AluOpType.add)

            # Noisy bias nb[b, :] = bias * Berr[idx[b]], then round-tripped
            # through DRAM to flatten onto partition 0 so the bias can be
            # PSUM-preloaded per batch element by the scalar engine.
            berr_sb = cpool.tile([b_shard, out], F32)
            nc.gpsimd.indirect_dma_start(
                out=berr_sb[:], out_offset=None, in_=berr_d[:],
                in_offset=bass.IndirectOffsetOnAxis(ap=idx_sb[:, :1], axis=0))
            ps_nb = spp.tile([b_shard, out], F32, space="PSUM")
            nc.tensor.matmul(ps_nb[:], lhsT=ones_row[:, :b_shard],
                             rhs=bias_v[:], start=True, stop=True)
            nb = cpool.tile([b_shard, out], BF16)
            nc.vector.tensor_tensor(out=nb[:], in0=ps_nb[:], in1=berr_sb[:],
                                    op=mybir.AluOpType.mult)
            nc.gpsimd.dma_start(out=nbstage_d[:], in_=nb[:])
            # Pre-load the DRAM output with the noisy bias rows (bf16->f32
            # cast in flight); the batched output DMAs then accumulate the
            # matmul results on top with the SDMA inline adder.
            nc.gpsimd.dma_start(out=out_d[:], in_=nbstage_d[:])

            # X transposed to [p, b] per sub-row j: x_js[j][p, b] = X[b, 4p+j]
            x_r = x_v[:].rearrange("b (p j) -> b j p", j=nj)
            x_js = []
            for j in range(nj):
                ps_x = spp.tile([P, b_shard], F32, space="PSUM")
                nc.tensor.transpose(ps_x[:], in_=x_r[:, j, :],
                                    identity=ident[:b_shard, :b_shard])
                xj = cpool.tile([P, b_shard], BF16, tag=f"xj{j}")
                nc.vector.tensor_copy(out=xj[:], in_=ps_x[:])
                x_js.append(xj)

            # Output rows accumulate on partition 0 in four groups, each
            # flushed by one batched DMA (keeps HWDGE lane usage bounded).
            rpg = b_shard // 4  # rows per group
            rowbufs = [cpool.tile([1, rpg * out], F32, tag=f"rb{g}",
                                  name=f"rowbuf{g}")
                       for g in range(4)]

            # The gathers alternate between the two HWDGE engines (SP and
            # Activation): each dynamic-offset AP lowering consumes a
            # register from the issuing engine's file (~22 available), so a
            # single engine cannot host all 32. Small rotating register sets
            # hold the offsets; Tile tracks register defs/uses for ordering.
            NREGS = 4
            dma_engs = [nc.sync, nc.scalar]
            eng_types = [mybir.EngineType.SP, mybir.EngineType.Activation]
            rregs = [[nc.alloc_register(et, f"ridx{e}_{r}")
                      for r in range(NREGS)]
                     for e, et in enumerate(eng_types)]

            # ---------------- main loop over batch elements ----------------
            hwdge_i = 0
            for b in range(b_shard):
                swdge = swdge_every and (b % swdge_every == swdge_every - 1)
                tmp = tpool.tile([P, chunk], BF16)
                if swdge:
                    # Indirect SWDGE gather with f32->bf16 cast in flight:
                    # halves this iteration's DVE multiply cost; the Q7
                    # descriptor generation rides the idle gpsimd engine.
                    werr_b = wpool.tile([P, chunk], BF16, name=f"werrb{b}",
                                        tag="werrb", bufs=3)
                    nc.gpsimd.indirect_dma_start(
                        out=werr_b[:], out_offset=None, in_=werr_d[:],
                        in_offset=bass.IndirectOffsetOnAxis(
                            ap=rows[:, b:b + 1], axis=0))
                    nc.vector.tensor_tensor(out=tmp[:], in0=werr_b[:],
                                            in1=w_bf[:],
                                            op=mybir.AluOpType.mult)
                else:
                    # Werr[idx[b]] row offset into the [pool*128, chunk]
                    # view, computed on the issuing engine's register file;
                    # the gather is a plain contiguous 1MB HWDGE DMA with a
                    # dynamic offset. Alternates between the two HWDGE
                    # engines (register files are per-engine and small).
                    e = hwdge_i % 2
                    hwdge_i += 1
                    eng = dma_engs[e]
                    rreg = rregs[e][(hwdge_i // 2) % NREGS]
                    eng.reg_load(rreg, idxt_sb[0:1, b:b + 1])
                    eng.reg_alu(rreg, rreg, P, mybir.AluOpType.mult)
                    val = bass.make_scalar_value(bass.RegisterHandles(rreg),
                                                 min_val=0,
                                                 max_val=(pool - 1) * P)
                    werr = wpool.tile([P, chunk], F32)
                    eng.dma_start(out=werr[:],
                                  in_=werr_d[bass.ds(val, P), :])
                    nc.vector.tensor_tensor(out=tmp[:], in0=werr[:],
                                            in1=w_sb[:],
                                            op=mybir.AluOpType.mult)

                pz = zpp.tile([1, out], F32, space="PSUM")
                for j in range(nj):
                    nc.tensor.matmul(pz[:], lhsT=x_js[j][:, b:b + 1],
                                     rhs=tmp[:, j * out:(j + 1) * out],
                                     start=(j == 0), stop=(j == nj - 1))
                nc.scalar.copy(
                    out=rowbufs[b // rpg][0:1, (b % rpg) * out:(b % rpg + 1) * out],
                    in_=pz[:])

            for g in range(4):
                nc.gpsimd.dma_start(out=out_d[g * rpg:(g + 1) * rpg, :],
                                    in_=rowbufs[g][:],
                                    accum_op=mybir.AluOpType.add)

    if legalize:
        # CoreSim can't digest the injected event-semaphore instructions
        # (it pre-computes fake sem updates); pass legalize=False to sim.
        _legalize_waits(nc)
    return nc


def make_in_maps(X, W, bias, Werr, Berr, idx, n_cores=N_CORES):
    b_shard = X.shape[0] // n_cores
    pool, in_, out = Werr.shape
    w_flat = np.ascontiguousarray(W.astype(np.float32)).reshape(P, -1)
    werr_flat = np.ascontiguousarray(Werr.astype(np.float32)).reshape(pool * P, -1)
    berr = np.ascontiguousarray(Berr.astype(np.float32))
    bias_row = np.ascontiguousarray(bias.astype(np.float32)).reshape(1, out)
    in_maps = []
    for c in range(n_cores):
        sl = slice(c * b_shard, (c + 1) * b_shard)
        idx_c = np.ascontiguousarray(idx[sl].astype(np.int32))
        in_maps.append({
            "X": np.ascontiguousarray(X[sl].astype(np.float32)),
            "W": w_flat,
            "bias": bias_row,
            "Werr": werr_flat,
            "Berr": berr,
            "idx": idx_c.reshape(-1, 1),
            "idxT": idx_c.reshape(1, -1),
        })
    return in_maps


_NC_CACHE = {}


def _get_nc():
    if "nc" not in _NC_CACHE:
        _NC_CACHE["nc"] = build_nc()
    return _NC_CACHE["nc"]


def run(X, W, bias, Werr, Berr, idx, trace=False, **kw):
    nc = _get_nc()
    in_maps = make_in_maps(X, W, bias, Werr, Berr, idx)
    res = run_bass_kernel_spmd(nc, in_maps, core_ids=list(range(N_CORES)),
                               trace=trace, **kw)
    out = np.concatenate([res.results[i]["out"] for i in range(N_CORES)], axis=0)
    return out, res


def kernel(X, W, bias, Werr, Berr, idx):
    out, _ = run(X, W, bias, Werr, Berr, idx)
    return out
